# revision 1
# baseline (speedup 1.0000x reference)
"""Trainium2 Bass kernel for nn_MultiHeadedAttention (B=4, S=1024, D=1024, H=16).

Sharding: 8 cores = 4 batches x 2 head-halves (8 heads each). The reference's
row-major reshape after [B,H,S,d] means output row r = h*64 + s//16 depends
only on head h, so head sharding needs no collective: each core computes a
[512, 1024] row-block of its batch's output.

Per-core pipeline (all matmuls contract on the partition dim):
  QT/KT = WxT.T @ XxT          -> [j, s] layout (head dims on partitions)
  V     = XvT.T @ WvT          -> [s, j] natural layout, augmented with a
                                  ones column per head (row 64 of PV psum
                                  then accumulates the softmax denominator)
  scoresT[k, q] = KT_h.T @ QT_h  (q in s16-major order so PV output lands in
                                  the layout the final reshape needs)
  wT = exp(0.125 * scoresT)      (mask is a no-op unless mask@mask.T has
                                  zeros; host checks and enables a penalty-add
                                  fallback path in that case)
  xT'[dd|sum, q] = V_aug.T @ wT  (accumulated over k tiles)
  lhsT = xT'[0:64] * (1/sum)     (DVE copy into x_block.T layout, 2 heads
                                  side by side)
  out  = lhsT.T @ WoT            -> [128 rows, 1024] per head pair, DMA'd out.

x / W_qkv / W_o are loaded as bf16 (host pre-cast): halves the 22MB/core DMA
footprint and lets FWL halve LDWEIGHTS. QK^T/softmax/PV stay f32r/f32.
x and W are loaded in per-128-row-chunk DMAs so the projections start after
the first 256KB instead of the first 2MB.
"""

import numpy as np
import ml_dtypes

import concourse.bass as bass
import concourse.bacc as bacc
import concourse.tile as tile
from concourse import mybir
from concourse.bass_utils import run_bass_kernel_spmd

F32 = mybir.dt.float32
F32R = mybir.dt.float32r
BF16 = mybir.dt.bfloat16
BF16_NP = ml_dtypes.bfloat16


B, S, D, H = 4, 1024, 1024, 16
d_head = D // H  # 64
HPC = 8          # heads per core
JC = HPC * d_head  # 512 columns of W per core

_cached = {}


def build_program(use_mask: bool, loop_n=None):
    nc = bacc.Bacc(None, target_bir_lowering=False, debug=False)

    xqT = nc.dram_tensor("xqT", [D, S], BF16, kind="ExternalInput").ap()
    xkT = nc.dram_tensor("xkT", [D, S], BF16, kind="ExternalInput").ap()
    xvT = nc.dram_tensor("xvT", [D, S], BF16, kind="ExternalInput").ap()
    wqT = nc.dram_tensor("wqT", [D, JC], BF16, kind="ExternalInput").ap()
    wkT = nc.dram_tensor("wkT", [D, JC], BF16, kind="ExternalInput").ap()
    wvT = nc.dram_tensor("wvT", [D, JC], BF16, kind="ExternalInput").ap()
    bq_col = nc.dram_tensor("bq_col", [128, 4], F32, kind="ExternalInput").ap()
    bk_col = nc.dram_tensor("bk_col", [128, 4], F32, kind="ExternalInput").ap()
    bv_bc = nc.dram_tensor("bv_bc", [128, JC], F32, kind="ExternalInput").ap()
    woT = nc.dram_tensor("woT", [D, D], BF16, kind="ExternalInput").ap()
    if use_mask:
        pen = nc.dram_tensor("pen", [S, S], F32, kind="ExternalInput").ap()
    out = nc.dram_tensor("out", [JC, D], F32, kind="ExternalOutput").ap()

    with tile.TileContext(nc) as tc:
        with (
            tc.tile_pool(name="xp", bufs=16) as xp,       # [128,1024] bf16
            tc.tile_pool(name="pp", bufs=8) as pp_p,      # mask-path pen tiles
            tc.tile_pool(name="wp", bufs=16) as wp,       # [128,512] bf16
            tc.tile_pool(name="qt", bufs=4) as qt_p,
            tc.tile_pool(name="kt", bufs=4) as kt_p,
            tc.tile_pool(name="va", bufs=8) as va_p,
            tc.tile_pool(name="wT", bufs=6) as wT_p,
            tc.tile_pool(name="lh", bufs=2) as lh_p,
            tc.tile_pool(name="outp", bufs=2) as outp,
            tc.tile_pool(name="small", bufs=6) as smallp,
            tc.tile_pool(name="psA", bufs=1, space="PSUM") as psA,
            tc.tile_pool(name="psB", bufs=4, space="PSUM") as psB,
        ):
          def emit_body():
            # per-128-row-chunk loads: dram [n*128, ncols] -> n tiles
            # [128, ncols]; dt accessor indexes the chunk.
            def load_chunks(dram, pool, tag, ncols, n, dt_ty, eng=None):
                eng = eng or nc.gpsimd
                ts = []
                for i in range(n):
                    t = pool.tile([128, ncols], dt_ty, tag=tag, name=tag)
                    src_ap = dram[i * 128:(i + 1) * 128, :]
                    if dt_ty == F32R:
                        src_ap = src_ap  # SWDGE casts f32 -> f32r in flight
                    eng.dma_start(t[:], src_ap)
                    ts.append(t)
                return lambda dt: ts[dt]

            pen_t = None
            if use_mask:
                # binary keep-mask, 8 chunk tiles (fallback path: slow but
                # correct; the fast path never loads these)
                pen_t = load_chunks(pen, pp_p, "pn", S, 8, F32R)

            def proj_jt(wt, xt, bias_sb, dst, jt):
                for st in range(2):
                    ps = psB.tile([128, 512], F32, tag="ps1", name="ps")[:]
                    for dt in range(8):
                        nc.tensor.matmul(
                            ps,
                            lhsT=wt(dt)[:, jt * 128:(jt + 1) * 128],
                            rhs=xt(dt)[:, st * 512:(st + 1) * 512],
                            start=(dt == 0),
                            stop=(dt == 7),
                        )
                    nc.vector.tensor_scalar_add(
                        dst[jt][:, st * 512:(st + 1) * 512], ps,
                        bias_sb[:, jt:jt + 1],
                    )

            def proj_qk(wt, xt, bias_sb, dst_pool):
                dst = [dst_pool.tile([128, S], F32R, tag="dst", name="dst")
                       for _ in range(4)]
                for jt in range(4):
                    proj_jt(wt, xt, bias_sb, dst, jt)
                return dst

            warm = smallp.tile([1, 8], F32, tag="warm", bufs=1)
            nc.vector.memset(warm[:], 0.0)
            nc.scalar.activation(warm[:], warm[:],
                                 mybir.ActivationFunctionType.Exp)

            wt_q = load_chunks(wqT, wp, "w", JC, 8, BF16, eng=nc.sync)
            xt_q = load_chunks(xqT, xp, "x", S, 8, BF16)
            wt_k = load_chunks(wkT, wp, "w", JC, 8, BF16, eng=nc.sync)
            xt_k = load_chunks(xkT, xp, "x", S, 8, BF16, eng=nc.sync)
            # biases ride the gpsimd ring so they never delay the K stream
            bq_sb = smallp.tile([128, 4], F32, tag="bias", bufs=2)
            nc.gpsimd.dma_start(bq_sb[:], bq_col[:])
            bk_sb = smallp.tile([128, 4], F32, tag="bias", bufs=2)
            nc.gpsimd.dma_start(bk_sb[:], bk_col[:])
            bv_sb = smallp.tile([128, JC], F32, tag="biasr", bufs=1)
            nc.gpsimd.dma_start(bv_sb[:], bv_bc[:])
            QT = proj_qk(wt_q, xt_q, bq_sb, qt_p)
            KT = proj_qk(wt_k, xt_k, bk_sb, kt_p)

            # ---- V projection -> V_aug [s, 8*65] (65th col per head = 1.0)
            wvt = load_chunks(wvT, wp, "w", JC, 8, BF16, eng=nc.sync)
            xvt = load_chunks(xvT, xp, "x", S, 8, BF16)
            VA = []
            for st in range(8):
                ps = psB.tile([128, 512], F32, tag="ps1")
                for dt in range(8):
                    nc.tensor.matmul(
                        ps[:],
                        lhsT=xvt(dt)[:, st * 128:(st + 1) * 128],
                        rhs=wvt(dt),
                        start=(dt == 0),
                        stop=(dt == 7),
                    )
                va = va_p.tile([128, 8 * 65], F32R)
                # only the 8 ones-columns need the fill; the rest is written
                # by the add below
                nc.vector.memset(
                    va[:].bitcast(F32).rearrange("p (h c) -> p h c",
                                                 h=8)[:, :, 64:65], 1.0)
                nc.vector.tensor_tensor(
                    va[:].rearrange("p (h c) -> p h c", h=8)[:, :, 0:64],
                    ps[:].rearrange("p (h c) -> p h c", h=8),
                    bv_sb[:].rearrange("p (h c) -> p h c", h=8),
                    op=mybir.AluOpType.add,
                )
                VA.append(va)

            # woT chunk tiles (reuse xp slots released by xq/xk tiles)
            wo_t = load_chunks(woT, xp, "x", D, 8, BF16)

            def QT_perm(hl, qch):
                # rhs [64, 512] with q in s16-major order:
                # col j reads s = q16*16 + s16, s16 = qch*8 + j//64, q16 = j%64
                tile_ = QT[hl // 2]
                po = (hl % 2) * 64
                ap = tile_[po:po + 64, :].rearrange("p (q s) -> p s q", s=16)
                return ap[:, qch * 8:(qch + 1) * 8, :]

            def KT_ap(hl, kt):
                tile_ = KT[hl // 2]
                po = (hl % 2) * 64
                return tile_[po:po + 64, kt * 128:(kt + 1) * 128]

            # ---- attention per head pair ----
            rc = smallp.tile([64, 1024], F32, tag="rc", bufs=1)
            rcb = smallp.tile([64, 1024], F32, tag="rcb", bufs=1)
            nc.vector.memset(rc[:], 1.0)  # rows 1-63 only feed the bcast AP

            PV_LAG = 2  # kt-steps the PV matmuls trail scores/exp

            def attention(p, hook_norm=None, hook_fp=None, lag=None):
                lag = PV_LAG if lag is None else lag
                hA, hB = 2 * p, 2 * p + 1
                pv = {}
                wstash = {}
                for step in range(8 + lag):
                    if step == 1 and hook_norm is not None:
                        hook_norm()
                    # fp chains are full-array (cannot overlap the ACT exp
                    # stream); emit them in the drain steps where the pair's
                    # exps are already done instead of mid-umbrella
                    if step == 8 and hook_fp is not None:
                        hook_fp()
                    if step < 8:
                        kt = step
                        # one [128,2048] psum tile: A scores in cols 0-1023,
                        # B in 1024-2047; A/B row-groups 0-63/64-127 overlap
                        # on PE, and ONE exp covers both heads (halves the
                        # per-instruction ACT init overhead + sync points)
                        sc = psA.tile([128, 2048], F32, tag="sc")
                        for qch in range(2):
                            nc.tensor.matmul(
                                sc[:, qch * 512:(qch + 1) * 512],
                                lhsT=KT_ap(hA, kt),
                                rhs=QT_perm(hA, qch),
                                start=True, stop=True,
                            )
                            nc.tensor.matmul(
                                sc[:, 1024 + qch * 512:1024 + (qch + 1) * 512],
                                lhsT=KT_ap(hB, kt),
                                rhs=QT_perm(hB, qch),
                                start=True, stop=True,
                            )
                        w = wT_p.tile([128, 2048], F32R, tag="wT")
                        nc.scalar.activation(w[:], sc[:],
                                             mybir.ActivationFunctionType.Exp,
                                             scale=0.125)
                        if use_mask:
                            # multiply by the 0/1 keep-mask (pen[k, q]) with
                            # the same s16-major q permutation as wT columns
                            pap = pen_t(kt).rearrange("p (q s) -> p s q", s=16)
                            for half in range(2):
                                wh = w[:, half * 1024:(half + 1) * 1024]
                                nc.vector.tensor_tensor(
                                    wh.rearrange("p (s q) -> p s q", s=16),
                                    wh.rearrange("p (s q) -> p s q", s=16),
                                    pap, op=mybir.AluOpType.mult,
                                )
                        wstash[kt] = w
                    if step >= lag:
                        kt = step - lag
                        w = wstash.pop(kt)
                        for i, (hl, half, qch) in enumerate(
                            [(hA, 0, 0), (hB, 1, 0), (hA, 0, 1), (hB, 1, 1)]
                        ):
                            if kt == 0:
                                pv[i] = psB.tile([65, 512], F32, tag="ps1", name="pv")
                            nc.tensor.matmul(
                                pv[i][:],
                                lhsT=VA[kt][:, hl * 65:hl * 65 + 65],
                                rhs=w[:, half * 1024 + qch * 512:
                                      half * 1024 + (qch + 1) * 512],
                                start=(kt == 0), stop=(kt == 7),
                            )
                return pv

            def tail_norm(p, pv):
                hA, hB = 2 * p, 2 * p + 1
                # normalize + shuffle into final-projection lhsT layout
                lh = lh_p.tile([128, 1024], BF16)
                for hloc, hl in enumerate((hA, hB)):
                    for qch in range(2):
                        i = hloc + 2 * qch
                        nc.vector.reciprocal(
                            rc[0:1, qch * 512:(qch + 1) * 512], pv[i][64:65, :])
                    nc.gpsimd.partition_broadcast(rcb[:], rc[:])
                    rcv = rcb[:].rearrange("p (s q) -> p s q", s=16)
                    for qch in range(2):
                        i = hloc + 2 * qch
                        src = pv[i][0:64, :].rearrange("p (s q) -> p s q", s=8)
                        for par, off in ((0, 0), (1, 64)):  # even/odd s16
                            # lh layout: [part, (ct 8)(head 2)(q16 64)] so the
                            # final matmul's lhsT tile ct is one contiguous
                            # 128-col block (walrus: stationary AP needs a
                            # single free dim)
                            dst = lh[off:off + 64, :].rearrange(
                                "p (c m) -> p c m", c=8
                            )[:, qch * 4:(qch + 1) * 4,
                              hloc * 64:(hloc + 1) * 64]
                            nc.vector.tensor_tensor(
                                dst,
                                src[:, par::2, :],
                                rcv[:, qch * 8 + par:qch * 8 + 8:2, :],
                                op=mybir.AluOpType.mult,
                            )

                return lh

            def tail_fp(p, lh):
                # final projection: out rows p*128 .. p*128+128
                ob = outp.tile([128, 1024], F32)
                for ot in range(2):
                    fp = psB.tile([128, 512], F32, tag="ps1")
                    for ct in range(8):
                        nc.tensor.matmul(
                            fp[:],
                            lhsT=lh[:, ct * 128:(ct + 1) * 128],
                            rhs=wo_t(ct)[:, ot * 512:(ot + 1) * 512],
                            start=(ct == 0), stop=(ct == 7),
                        )
                    nc.vector.tensor_copy(
                        ob[:, ot * 512:(ot + 1) * 512], fp[:])
                nc.sync.dma_start(out[p * 128:(p + 1) * 128, :], ob[:])

            # software-pipeline: pair p-1's norm (DVE) is emitted in p-1's
            # own drain (after its last PV), keeping the next umbrella free
            # of DVE work and freeing its psum before pair p allocates; the
            # fp chains still ride pair p's drain via the hook
            pending = None
            for p in range(4):
                hf = None
                if pending is not None:
                    pp, plh = pending

                    def hf(pp=pp, plh=plh):
                        tail_fp(pp, plh)

                pv = attention(p, None, hf)
                pending = (p, tail_norm(p, pv))
            pp, plh = pending
            tail_fp(pp, plh)

          if loop_n is None:
              emit_body()
          else:
              with tc.For_i(0, loop_n):
                  emit_body()

    nc.compile()
    return nc


def make_in_maps(query, key, value, mask, Wq, bq, Wk, bk, Wv, bv, Wo,
                 pen_b=None):
    woT = np.ascontiguousarray(Wo.T).astype(BF16_NP)
    maps = []
    for c in range(8):
        b, hf = c // 2, c % 2
        sl = slice(hf * JC, (hf + 1) * JC)
        m = {
            "xqT": np.ascontiguousarray(query[b].T).astype(BF16_NP),
            "xkT": np.ascontiguousarray(key[b].T).astype(BF16_NP),
            "xvT": np.ascontiguousarray(value[b].T).astype(BF16_NP),
            "wqT": np.ascontiguousarray(Wq[sl].T).astype(BF16_NP),
            "wkT": np.ascontiguousarray(Wk[sl].T).astype(BF16_NP),
            "wvT": np.ascontiguousarray(Wv[sl].T).astype(BF16_NP),
            "bq_col": np.ascontiguousarray(bq[sl].reshape(4, 128).T),
            "bk_col": np.ascontiguousarray(bk[sl].reshape(4, 128).T),
            "bv_bc": np.ascontiguousarray(
                np.broadcast_to(bv[sl].reshape(1, JC), (128, JC))),
            "woT": woT,
        }
        if pen_b is not None:
            m["pen"] = pen_b[b]
        maps.append(m)
    return maps


def kernel(query, key, value, mask, Wq, bq, Wk, bk, Wv, bv, Wo):
    query = np.asarray(query, np.float32)
    key = np.asarray(key, np.float32)
    value = np.asarray(value, np.float32)
    mask = np.asarray(mask, np.float32)

    m2d = mask[0]  # [B, S, 64]
    mm = np.stack([m2d[b] @ m2d[b].T for b in range(B)])  # [B, S, S]
    use_mask = bool((mm == 0).any())
    pen_b = None
    if use_mask:
        pen_b = np.where(mm == 0, np.float32(0.0), np.float32(1.0))
        pen_b = np.ascontiguousarray(pen_b, np.float32)

    if use_mask not in _cached:
        _cached[use_mask] = build_program(use_mask)
    nc = _cached[use_mask]

    in_maps = make_in_maps(query, key, value, mask,
                           np.asarray(Wq, np.float32), np.asarray(bq, np.float32),
                           np.asarray(Wk, np.float32), np.asarray(bk, np.float32),
                           np.asarray(Wv, np.float32), np.asarray(bv, np.float32),
                           np.asarray(Wo, np.float32), pen_b)
    res = run_bass_kernel_spmd(nc, in_maps, list(range(8)))

    out = np.empty((B, S, D), np.float32)
    for c in range(8):
        b, hf = c // 2, c % 2
        out[b, hf * JC:(hf + 1) * JC, :] = res.results[c]["out"]
    return out



# revision 2
# speedup vs baseline: 4.2339x; 4.2339x over previous
"""Trainium2 Bass kernel for nn_MultiHeadedAttention (B=4, S=1024, D=1024, H=16).

Sharding: 8 cores = 4 batches x 2 head-halves (8 heads each). The reference's
row-major reshape after [B,H,S,d] means output row r = h*64 + s//16 depends
only on head h, so head sharding needs no collective: each core computes a
[512, 1024] row-block of its batch's output.

Per-core pipeline (all matmuls contract on the partition dim):
  QT/KT = WxT.T @ XxT          -> [j, s] layout (head dims on partitions),
                                  stored bf16 so QK^T runs as a bf16 matmul
  V     = XvT.T @ WvT          -> [s, j] natural layout, augmented with a
                                  ones column per head (row 64 of PV psum
                                  then accumulates the softmax denominator)
  scoresT[k, q] = KT_h.T @ QT_h  (q in s16-major order so PV output lands in
                                  the layout the final reshape needs; one
                                  [128,1024] psum tile per (kt, head),
                                  double-buffered so exp never blocks the
                                  next scores matmul)
  wT = exp(0.125 * scoresT)      (mask is a no-op unless mask@mask.T has
                                  zeros; host checks and enables a penalty-add
                                  fallback path in that case)
  xT'[dd|sum, q] = V_aug.T @ wT  (accumulated over k tiles)
  lhsT = xT'[0:64] * (1/sum)     (DVE copy into x_block.T layout, 2 heads
                                  side by side)
  out  = lhsT.T @ WoT            -> [128 rows, 1024] per head pair, DMA'd out.

x / W_qkv / W_o are loaded as bf16 (host pre-cast): halves the 22MB/core DMA
footprint. Matmul loops are ordered so consecutive instructions share the
stationary (lhsT) operand wherever possible, halving PE weight loads.
softmax/PV stay f32/f32r.
"""

import numpy as np
import ml_dtypes

import concourse.bass as bass
import concourse.bacc as bacc
import concourse.tile as tile
from concourse import mybir
from concourse.bass_utils import run_bass_kernel_spmd

F32 = mybir.dt.float32
F32R = mybir.dt.float32r
BF16 = mybir.dt.bfloat16
BF16_NP = ml_dtypes.bfloat16


B, S, D, H = 4, 1024, 1024, 16
d_head = D // H  # 64
HPC = 8          # heads per core
JC = HPC * d_head  # 512 columns of W per core

_cached = {}


def build_program(use_mask: bool, loop_n=None):
    nc = bacc.Bacc(None, target_bir_lowering=False, debug=False)

    xqT = nc.dram_tensor("xqT", [D, S], BF16, kind="ExternalInput").ap()
    xkT = nc.dram_tensor("xkT", [D, S], BF16, kind="ExternalInput").ap()
    xvT = nc.dram_tensor("xvT", [D, S], BF16, kind="ExternalInput").ap()
    wqT = nc.dram_tensor("wqT", [D, JC], BF16, kind="ExternalInput").ap()
    wkT = nc.dram_tensor("wkT", [D, JC], BF16, kind="ExternalInput").ap()
    wvT = nc.dram_tensor("wvT", [D, JC], BF16, kind="ExternalInput").ap()
    bq_col = nc.dram_tensor("bq_col", [128, 4], F32, kind="ExternalInput").ap()
    bk_col = nc.dram_tensor("bk_col", [128, 4], F32, kind="ExternalInput").ap()
    bv_bc = nc.dram_tensor("bv_bc", [128, JC], F32, kind="ExternalInput").ap()
    woT = nc.dram_tensor("woT", [D, D], BF16, kind="ExternalInput").ap()
    if use_mask:
        pen = nc.dram_tensor("pen", [S, S], F32, kind="ExternalInput").ap()
    out = nc.dram_tensor("out", [JC, D], F32, kind="ExternalOutput").ap()

    with tile.TileContext(nc) as tc:
        with (
            tc.tile_pool(name="xp", bufs=16) as xp,       # [128,1024] bf16
            tc.tile_pool(name="pp", bufs=8) as pp_p,      # mask-path pen tiles
            tc.tile_pool(name="wp", bufs=16) as wp,       # [128,512] bf16
            tc.tile_pool(name="qt", bufs=4) as qt_p,
            tc.tile_pool(name="kt", bufs=4) as kt_p,
            tc.tile_pool(name="va", bufs=8) as va_p,
            tc.tile_pool(name="wT", bufs=6) as wT_p,
            tc.tile_pool(name="lh", bufs=2) as lh_p,
            tc.tile_pool(name="outp", bufs=2) as outp,
            tc.tile_pool(name="small", bufs=6) as smallp,
            tc.tile_pool(name="psA", bufs=2, space="PSUM") as psA,
            tc.tile_pool(name="psB", bufs=4, space="PSUM") as psB,
        ):
          def emit_body():
            # per-128-row-chunk loads: dram [n*128, ncols] -> n tiles
            # [128, ncols]; dt accessor indexes the chunk.
            def load_chunks(dram, pool, tag, ncols, n, dt_ty, eng=None):
                eng = eng or nc.gpsimd
                ts = []
                for i in range(n):
                    t = pool.tile([128, ncols], dt_ty, tag=tag, name=tag)
                    src_ap = dram[i * 128:(i + 1) * 128, :]
                    eng.dma_start(t[:], src_ap)
                    ts.append(t)
                return lambda dt: ts[dt]

            pen_t = None
            if use_mask:
                # binary keep-mask, 8 chunk tiles (fallback path: slow but
                # correct; the fast path never loads these)
                pen_t = load_chunks(pen, pp_p, "pn", S, 8, F32R)

            def proj_jt(wt, xt, bias_sb, dst, jt):
                # dt-outer with both st psum tiles live: each weight chunk
                # (lhsT) feeds 2 consecutive matmuls, halving PE weight loads
                ps = [psB.tile([128, 512], F32, tag="ps1", name="ps")
                      for _ in range(2)]
                for dt in range(8):
                    for st in range(2):
                        nc.tensor.matmul(
                            ps[st][:],
                            lhsT=wt(dt)[:, jt * 128:(jt + 1) * 128],
                            rhs=xt(dt)[:, st * 512:(st + 1) * 512],
                            start=(dt == 0),
                            stop=(dt == 7),
                        )
                for st in range(2):
                    nc.vector.tensor_scalar_add(
                        dst[jt][:, st * 512:(st + 1) * 512], ps[st][:],
                        bias_sb[:, jt:jt + 1],
                    )

            def proj_qk(wt, xt, bias_sb, dst_pool):
                dst = [dst_pool.tile([128, S], BF16, tag="dst", name="dst")
                       for _ in range(4)]
                for jt in range(4):
                    proj_jt(wt, xt, bias_sb, dst, jt)
                return dst

            warm = smallp.tile([1, 8], F32, tag="warm", bufs=1)
            nc.vector.memset(warm[:], 0.0)
            nc.scalar.activation(warm[:], warm[:],
                                 mybir.ActivationFunctionType.Exp)

            wt_q = load_chunks(wqT, wp, "w", JC, 8, BF16, eng=nc.sync)
            xt_q = load_chunks(xqT, xp, "x", S, 8, BF16)
            wt_k = load_chunks(wkT, wp, "w", JC, 8, BF16, eng=nc.sync)
            xt_k = load_chunks(xkT, xp, "x", S, 8, BF16, eng=nc.sync)
            # biases ride the gpsimd ring so they never delay the K stream
            bq_sb = smallp.tile([128, 4], F32, tag="bias", bufs=2)
            nc.gpsimd.dma_start(bq_sb[:], bq_col[:])
            bk_sb = smallp.tile([128, 4], F32, tag="bias", bufs=2)
            nc.gpsimd.dma_start(bk_sb[:], bk_col[:])
            bv_sb = smallp.tile([128, JC], F32, tag="biasr", bufs=1)
            nc.gpsimd.dma_start(bv_sb[:], bv_bc[:])
            QT = proj_qk(wt_q, xt_q, bq_sb, qt_p)
            KT = proj_qk(wt_k, xt_k, bk_sb, kt_p)

            # ---- V projection -> V_aug [s, 8*65] (65th col per head = 1.0)
            wvt = load_chunks(wvT, wp, "w", JC, 8, BF16, eng=nc.sync)
            xvt = load_chunks(xvT, xp, "x", S, 8, BF16)
            VA = []
            for st in range(8):
                ps = psB.tile([128, 512], F32, tag="ps1")
                for dt in range(8):
                    nc.tensor.matmul(
                        ps[:],
                        lhsT=xvt(dt)[:, st * 128:(st + 1) * 128],
                        rhs=wvt(dt),
                        start=(dt == 0),
                        stop=(dt == 7),
                    )
                va = va_p.tile([128, 8 * 65], F32R)
                # only the 8 ones-columns need the fill; the rest is written
                # by the add below
                nc.vector.memset(
                    va[:].bitcast(F32).rearrange("p (h c) -> p h c",
                                                 h=8)[:, :, 64:65], 1.0)
                nc.vector.tensor_tensor(
                    va[:].rearrange("p (h c) -> p h c", h=8)[:, :, 0:64],
                    ps[:].rearrange("p (h c) -> p h c", h=8),
                    bv_sb[:].rearrange("p (h c) -> p h c", h=8),
                    op=mybir.AluOpType.add,
                )
                VA.append(va)

            # woT chunk tiles (reuse xp slots released by xq/xk tiles)
            wo_t = load_chunks(woT, xp, "x", D, 8, BF16)

            def QT_perm(hl, qch):
                # rhs [64, 512] with q in s16-major order:
                # col j reads s = q16*16 + s16, s16 = qch*8 + j//64, q16 = j%64
                tile_ = QT[hl // 2]
                po = (hl % 2) * 64
                ap = tile_[po:po + 64, :].rearrange("p (q s) -> p s q", s=16)
                return ap[:, qch * 8:(qch + 1) * 8, :]

            def KT_ap(hl, kt):
                tile_ = KT[hl // 2]
                po = (hl % 2) * 64
                return tile_[po:po + 64, kt * 128:(kt + 1) * 128]

            # ---- attention per head pair ----
            rc = smallp.tile([64, 1024], F32, tag="rc", bufs=1)
            rcb = smallp.tile([64, 1024], F32, tag="rcb", bufs=1)
            nc.vector.memset(rc[:], 1.0)  # rows 1-63 only feed the bcast AP

            PV_LAG = 3  # half-steps the PV matmuls trail scores/exp

            def attention(p, hook_norm=None, hook_fp=None, lag=None):
                lag = PV_LAG if lag is None else lag
                hA, hB = 2 * p, 2 * p + 1
                nsteps = 16  # (kt, head) half-steps
                pv = {}
                wstash = {}
                for step in range(nsteps + lag):
                    if step == 1 and hook_norm is not None:
                        hook_norm()
                    # fp chains are full-array (cannot overlap the ACT exp
                    # stream); emit them in the drain steps where the pair's
                    # exps are already done instead of mid-umbrella
                    if step == nsteps and hook_fp is not None:
                        hook_fp()
                    if step < nsteps:
                        kt, hloc = divmod(step, 2)
                        hl = hA if hloc == 0 else hB
                        # per-head [128,1024] psum tile, double-buffered:
                        # exp of step s overlaps scores of step s+1
                        sc = psA.tile([128, 1024], F32, tag="sc")
                        for qch in range(2):
                            nc.tensor.matmul(
                                sc[:, qch * 512:(qch + 1) * 512],
                                lhsT=KT_ap(hl, kt),
                                rhs=QT_perm(hl, qch),
                                start=True, stop=True,
                            )
                        w = wT_p.tile([128, 1024], F32R, tag="wT")
                        nc.scalar.activation(w[:], sc[:],
                                             mybir.ActivationFunctionType.Exp,
                                             scale=0.125)
                        if use_mask:
                            # multiply by the 0/1 keep-mask (pen[k, q]) with
                            # the same s16-major q permutation as wT columns
                            pap = pen_t(kt).rearrange("p (q s) -> p s q", s=16)
                            nc.vector.tensor_tensor(
                                w[:].rearrange("p (s q) -> p s q", s=16),
                                w[:].rearrange("p (s q) -> p s q", s=16),
                                pap, op=mybir.AluOpType.mult,
                            )
                        wstash[step] = w
                    if step >= lag:
                        kt, hloc = divmod(step - lag, 2)
                        hl = hA if hloc == 0 else hB
                        w = wstash.pop(step - lag)
                        for qch in range(2):
                            i = hloc + 2 * qch
                            if kt == 0:
                                pv[i] = psB.tile([65, 512], F32, tag="ps1",
                                                 name="pv")
                            nc.tensor.matmul(
                                pv[i][:],
                                lhsT=VA[kt][:, hl * 65:hl * 65 + 65],
                                rhs=w[:, qch * 512:(qch + 1) * 512],
                                start=(kt == 0), stop=(kt == 7),
                            )
                return pv

            def tail_norm(p, pv):
                hA, hB = 2 * p, 2 * p + 1
                # normalize + shuffle into final-projection lhsT layout
                lh = lh_p.tile([128, 1024], BF16)
                for hloc, hl in enumerate((hA, hB)):
                    for qch in range(2):
                        i = hloc + 2 * qch
                        nc.vector.reciprocal(
                            rc[0:1, qch * 512:(qch + 1) * 512], pv[i][64:65, :])
                    nc.gpsimd.partition_broadcast(rcb[:], rc[:])
                    rcv = rcb[:].rearrange("p (s q) -> p s q", s=16)
                    for qch in range(2):
                        i = hloc + 2 * qch
                        src = pv[i][0:64, :].rearrange("p (s q) -> p s q", s=8)
                        for par, off in ((0, 0), (1, 64)):  # even/odd s16
                            # lh layout: [part, (ct 8)(head 2)(q16 64)] so the
                            # final matmul's lhsT tile ct is one contiguous
                            # 128-col block (walrus: stationary AP needs a
                            # single free dim)
                            dst = lh[off:off + 64, :].rearrange(
                                "p (c m) -> p c m", c=8
                            )[:, qch * 4:(qch + 1) * 4,
                              hloc * 64:(hloc + 1) * 64]
                            nc.vector.tensor_tensor(
                                dst,
                                src[:, par::2, :],
                                rcv[:, qch * 8 + par:qch * 8 + 8:2, :],
                                op=mybir.AluOpType.mult,
                            )

                return lh

            def tail_fp(p, lh):
                # final projection: out rows p*128 .. p*128+128
                # ct-outer with both ot psum tiles live: each lh chunk (lhsT)
                # feeds 2 consecutive matmuls, halving PE weight loads
                ob = outp.tile([128, 1024], F32)
                fp = [psB.tile([128, 512], F32, tag="ps1", name="fp")
                      for _ in range(2)]
                for ct in range(8):
                    for ot in range(2):
                        nc.tensor.matmul(
                            fp[ot][:],
                            lhsT=lh[:, ct * 128:(ct + 1) * 128],
                            rhs=wo_t(ct)[:, ot * 512:(ot + 1) * 512],
                            start=(ct == 0), stop=(ct == 7),
                        )
                for ot in range(2):
                    nc.vector.tensor_copy(
                        ob[:, ot * 512:(ot + 1) * 512], fp[ot][:])
                nc.sync.dma_start(out[p * 128:(p + 1) * 128, :], ob[:])

            # software-pipeline: pair p-1's norm (DVE) is emitted in p-1's
            # own drain (after its last PV), keeping the next umbrella free
            # of DVE work and freeing its psum before pair p allocates; the
            # fp chains still ride pair p's drain via the hook
            pending = None
            for p in range(4):
                hf = None
                if pending is not None:
                    pp, plh = pending

                    def hf(pp=pp, plh=plh):
                        tail_fp(pp, plh)

                pv = attention(p, None, hf)
                pending = (p, tail_norm(p, pv))
            pp, plh = pending
            tail_fp(pp, plh)

          if loop_n is None:
              emit_body()
          else:
              with tc.For_i(0, loop_n):
                  emit_body()

    nc.compile()
    return nc


def make_in_maps(query, key, value, mask, Wq, bq, Wk, bk, Wv, bv, Wo,
                 pen_b=None):
    woT = np.ascontiguousarray(Wo.T).astype(BF16_NP)
    maps = []
    for c in range(8):
        b, hf = c // 2, c % 2
        sl = slice(hf * JC, (hf + 1) * JC)
        m = {
            "xqT": np.ascontiguousarray(query[b].T).astype(BF16_NP),
            "xkT": np.ascontiguousarray(key[b].T).astype(BF16_NP),
            "xvT": np.ascontiguousarray(value[b].T).astype(BF16_NP),
            "wqT": np.ascontiguousarray(Wq[sl].T).astype(BF16_NP),
            "wkT": np.ascontiguousarray(Wk[sl].T).astype(BF16_NP),
            "wvT": np.ascontiguousarray(Wv[sl].T).astype(BF16_NP),
            "bq_col": np.ascontiguousarray(bq[sl].reshape(4, 128).T),
            "bk_col": np.ascontiguousarray(bk[sl].reshape(4, 128).T),
            "bv_bc": np.ascontiguousarray(
                np.broadcast_to(bv[sl].reshape(1, JC), (128, JC))),
            "woT": woT,
        }
        if pen_b is not None:
            m["pen"] = pen_b[b]
        maps.append(m)
    return maps


def kernel(query, key, value, mask, Wq, bq, Wk, bk, Wv, bv, Wo):
    query = np.asarray(query, np.float32)
    key = np.asarray(key, np.float32)
    value = np.asarray(value, np.float32)
    mask = np.asarray(mask, np.float32)

    m2d = mask[0]  # [B, S, 64]
    mm = np.stack([m2d[b] @ m2d[b].T for b in range(B)])  # [B, S, S]
    use_mask = bool((mm == 0).any())
    pen_b = None
    if use_mask:
        pen_b = np.where(mm == 0, np.float32(0.0), np.float32(1.0))
        pen_b = np.ascontiguousarray(pen_b, np.float32)

    if use_mask not in _cached:
        _cached[use_mask] = build_program(use_mask)
    nc = _cached[use_mask]

    in_maps = make_in_maps(query, key, value, mask,
                           np.asarray(Wq, np.float32), np.asarray(bq, np.float32),
                           np.asarray(Wk, np.float32), np.asarray(bk, np.float32),
                           np.asarray(Wv, np.float32), np.asarray(bv, np.float32),
                           np.asarray(Wo, np.float32), pen_b)
    res = run_bass_kernel_spmd(nc, in_maps, list(range(8)))

    out = np.empty((B, S, D), np.float32)
    for c in range(8):
        b, hf = c // 2, c % 2
        out[b, hf * JC:(hf + 1) * JC, :] = res.results[c]["out"]
    return out


# revision 3
# speedup vs baseline: 5.7657x; 1.3618x over previous
"""Trainium2 Bass kernel for nn_MultiHeadedAttention (B=4, S=1024, D=1024, H=16).

Sharding: 8 cores = 4 batches x 2 head-halves (8 heads each). The reference's
row-major reshape after [B,H,S,d] means output row r = h*64 + s//16 depends
only on head h, so head sharding needs no collective: each core computes a
[512, 1024] row-block of its batch's output.

Per-core pipeline (all matmuls contract on the partition dim):
  QT = WqT.T @ XqT   -> [j, s], stored bf16 PRE-PERMUTED (cols in the
                        s16-major order the scores matmul wants) so the PE's
                        moving reads are contiguous: HW charges ~2-5x for
                        strided moving APs (worst for 2-byte dtypes), which
                        the CoreSim cost model does not model.
  KT = WkT.T @ XkT   -> per-head [128, s] bf16 tiles with the other head's
                        64 partitions zeroed: scores then run at contract=128
                        (HW charges ~2.2x for 64-contract tiles) with the
                        full 2-head QT tile as rhs (other head annihilated
                        by the zeros).
  V  = XvT.T @ WvT   -> [s, j] natural layout, augmented with a ones column
                        per head (row 64 of PV psum then accumulates the
                        softmax denominator).
  scoresT[k, q] = KTz_h.T @ QTp  per (kt, head): one [128,1024] psum tile,
                        double-buffered (psA bufs=2) so exp(step s) overlaps
                        the scores matmuls of step s+1.
  wT = exp(0.125 * scoresT)   (mask is a no-op unless mask@mask.T has zeros;
                        host checks and enables a penalty-mult fallback)
  xT'[dd|sum, q] = V_aug.T @ wT  (accumulated over k tiles, f32r)
  lhsT = xT'[0:64] * (1/sum)     (DVE into x_block.T layout)
  out  = lhsT.T @ WoT  -> [128 rows, 1024] per head pair, DMA'd out.

x / W_qkv / W_o are loaded as bf16 (host pre-cast). Measured on HW (loop-
slope bench): 211.0us vs 231.4us baseline; CoreSim predicts 134us (it
misses the strided-AP and 64-contract matmul penalties this layout avoids).
"""

import numpy as np
import ml_dtypes

import concourse.bass as bass
import concourse.bacc as bacc
import concourse.tile as tile
from concourse import mybir
from concourse.bass_utils import run_bass_kernel_spmd

F32 = mybir.dt.float32
F32R = mybir.dt.float32r
BF16 = mybir.dt.bfloat16
BF16_NP = ml_dtypes.bfloat16


B, S, D, H = 4, 1024, 1024, 16
d_head = D // H  # 64
HPC = 8          # heads per core
JC = HPC * d_head  # 512 columns of W per core

_cached = {}


def build_program(use_mask: bool, loop_n=None):
    nc = bacc.Bacc(None, target_bir_lowering=False, debug=False)

    xqT = nc.dram_tensor("xqT", [D, S], BF16, kind="ExternalInput").ap()
    xkT = nc.dram_tensor("xkT", [D, S], BF16, kind="ExternalInput").ap()
    xvT = nc.dram_tensor("xvT", [D, S], BF16, kind="ExternalInput").ap()
    wqT = nc.dram_tensor("wqT", [D, JC], BF16, kind="ExternalInput").ap()
    wkT = nc.dram_tensor("wkT", [D, JC], BF16, kind="ExternalInput").ap()
    wvT = nc.dram_tensor("wvT", [D, JC], BF16, kind="ExternalInput").ap()
    bq_col = nc.dram_tensor("bq_col", [128, 4], F32, kind="ExternalInput").ap()
    bk_col = nc.dram_tensor("bk_col", [128, 4], F32, kind="ExternalInput").ap()
    bv_bc = nc.dram_tensor("bv_bc", [128, JC], F32, kind="ExternalInput").ap()
    woT = nc.dram_tensor("woT", [D, D], BF16, kind="ExternalInput").ap()
    if use_mask:
        pen = nc.dram_tensor("pen", [S, S], F32, kind="ExternalInput").ap()
    out = nc.dram_tensor("out", [JC, D], F32, kind="ExternalOutput").ap()

    with tile.TileContext(nc) as tc:
        with (
            tc.tile_pool(name="xp", bufs=16) as xp,       # [128,1024] bf16
            tc.tile_pool(name="pp", bufs=8) as pp_p,      # mask-path pen tiles
            tc.tile_pool(name="wp", bufs=16) as wp,       # [128,512] bf16
            tc.tile_pool(name="qt", bufs=4) as qt_p,
            tc.tile_pool(name="kt", bufs=8) as kt_p,
            tc.tile_pool(name="va", bufs=8) as va_p,
            tc.tile_pool(name="wT", bufs=8) as wT_p,
            tc.tile_pool(name="lh", bufs=2) as lh_p,
            tc.tile_pool(name="outp", bufs=2) as outp,
            tc.tile_pool(name="small", bufs=6) as smallp,
            tc.tile_pool(name="psA", bufs=2, space="PSUM") as psA,
            tc.tile_pool(name="psB", bufs=4, space="PSUM") as psB,
        ):
          def emit_body():
            # per-128-row-chunk loads: dram [n*128, ncols] -> n tiles
            # [128, ncols]; dt accessor indexes the chunk.
            def load_chunks(dram, pool, tag, ncols, n, dt_ty, eng=None):
                eng = eng or nc.gpsimd
                ts = []
                for i in range(n):
                    t = pool.tile([128, ncols], dt_ty, tag=tag, name=tag)
                    eng.dma_start(t[:], dram[i * 128:(i + 1) * 128, :])
                    ts.append(t)
                return lambda dt: ts[dt]

            pen_t = None
            if use_mask:
                # binary keep-mask, 8 chunk tiles (fallback path: slow but
                # correct; the fast path never loads these)
                pen_t = load_chunks(pen, pp_p, "pn", S, 8, F32R)

            def q_dst_ap(dst, jt, st):
                # permuted store: QTp col = qch*512 + s16l*64 + q16 so the
                # scores matmul's moving reads are contiguous. psum natural
                # col q' = q16h*16 + qch*8 + s16l (q16 = st*32 + q16h).
                return dst[jt][:, :].rearrange(
                    "p (qc s8 q16) -> p qc s8 q16", qc=2, s8=8
                )[:, :, :, st * 32:(st + 1) * 32]

            def q_src_ap(ps):
                return ps[:].rearrange("p (a b c) -> p b c a", a=32, b=2)

            warm = smallp.tile([1, 8], F32, tag="warm", bufs=1)
            nc.vector.memset(warm[:], 0.0)
            nc.scalar.activation(warm[:], warm[:],
                                 mybir.ActivationFunctionType.Exp)

            wt_q = load_chunks(wqT, wp, "w", JC, 8, BF16, eng=nc.sync)
            xt_q = load_chunks(xqT, xp, "x", S, 8, BF16)
            wt_k = load_chunks(wkT, wp, "w", JC, 8, BF16, eng=nc.sync)
            xt_k = load_chunks(xkT, xp, "x", S, 8, BF16, eng=nc.sync)
            # biases ride the gpsimd ring so they never delay the K stream
            bq_sb = smallp.tile([128, 4], F32, tag="bias", bufs=2)
            nc.gpsimd.dma_start(bq_sb[:], bq_col[:])
            bk_sb = smallp.tile([128, 4], F32, tag="bias", bufs=2)
            nc.gpsimd.dma_start(bk_sb[:], bk_col[:])
            bv_sb = smallp.tile([128, JC], F32, tag="biasr", bufs=1)
            nc.gpsimd.dma_start(bv_sb[:], bv_bc[:])

            # ---- Q projection -> QT tiles, bf16, pre-permuted columns ----
            QT = [qt_p.tile([128, S], BF16, tag="dst", name="dst")
                  for _ in range(4)]
            for jt in range(4):
                ps = [psB.tile([128, 512], F32, tag="ps1", name="ps")
                      for _ in range(2)]
                for dt in range(8):
                    for st in range(2):
                        nc.tensor.matmul(
                            ps[st][:],
                            lhsT=wt_q(dt)[:, jt * 128:(jt + 1) * 128],
                            rhs=xt_q(dt)[:, st * 512:(st + 1) * 512],
                            start=(dt == 0), stop=(dt == 7))
                for st in range(2):
                    nc.vector.tensor_scalar_add(
                        q_dst_ap(QT, jt, st), q_src_ap(ps[st]),
                        bq_sb[:, jt:jt + 1])

            # ---- K projection -> per-head zero-padded KT tiles, bf16 ----
            KT = [kt_p.tile([128, S], BF16, tag="dstz", name="ktz", bufs=8)
                  for _ in range(8)]
            for h in range(8):
                po = (h % 2) * 64
                nc.vector.memset(KT[h][64 - po:128 - po, :], 0.0)
            for jt in range(4):
                ps = [psB.tile([128, 512], F32, tag="ps1", name="ps")
                      for _ in range(2)]
                for dt in range(8):
                    for st in range(2):
                        nc.tensor.matmul(
                            ps[st][:],
                            lhsT=wt_k(dt)[:, jt * 128:(jt + 1) * 128],
                            rhs=xt_k(dt)[:, st * 512:(st + 1) * 512],
                            start=(dt == 0), stop=(dt == 7))
                for st in range(2):
                    for hloc in range(2):
                        po = hloc * 64
                        nc.vector.tensor_scalar_add(
                            KT[2 * jt + hloc][po:po + 64,
                                              st * 512:(st + 1) * 512],
                            ps[st][po:po + 64, :],
                            bk_sb[po:po + 64, jt:jt + 1])

            # ---- V projection -> V_aug [s, 8*65] (65th col per head = 1.0)
            wvt = load_chunks(wvT, wp, "w", JC, 8, BF16, eng=nc.sync)
            xvt = load_chunks(xvT, xp, "x", S, 8, BF16)
            VA = []
            for st in range(8):
                ps = psB.tile([128, 512], F32, tag="ps1")
                for dt in range(8):
                    nc.tensor.matmul(
                        ps[:],
                        lhsT=xvt(dt)[:, st * 128:(st + 1) * 128],
                        rhs=wvt(dt),
                        start=(dt == 0), stop=(dt == 7))
                va = va_p.tile([128, 8 * 65], F32R)
                # only the 8 ones-columns need the fill; the rest is written
                # by the add below
                nc.vector.memset(
                    va[:].bitcast(F32).rearrange("p (h c) -> p h c",
                                                 h=8)[:, :, 64:65], 1.0)
                nc.vector.tensor_tensor(
                    va[:].rearrange("p (h c) -> p h c", h=8)[:, :, 0:64],
                    ps[:].rearrange("p (h c) -> p h c", h=8),
                    bv_sb[:].rearrange("p (h c) -> p h c", h=8),
                    op=mybir.AluOpType.add)
                VA.append(va)

            # woT chunk tiles (reuse xp slots released by xq/xk tiles)
            wo_t = load_chunks(woT, xp, "x", D, 8, BF16)

            def QT_full(hl, qch):
                # full 128 partitions: the other head's rows hit the
                # zero-padded half of the stationary K tile
                return QT[hl // 2][:, qch * 512:(qch + 1) * 512]

            def KT_ap(hl, kt):
                return KT[hl][:, kt * 128:(kt + 1) * 128]

            # ---- attention per head pair ----
            rc = smallp.tile([64, 1024], F32, tag="rc", bufs=1)
            rcb = smallp.tile([64, 1024], F32, tag="rcb", bufs=1)
            nc.vector.memset(rc[:], 1.0)  # rows 1-63 only feed the bcast AP

            PV_LAG = 3  # half-steps the PV matmuls trail scores/exp

            def attention(p, hook_fp, lag):
                hA, hB = 2 * p, 2 * p + 1
                nsteps = 16  # (kt, head) half-steps
                pv = {}
                wstash = {}
                for step in range(nsteps + lag):
                    # fp chains ride the drain steps, where the pair's exps
                    # are already done
                    if step == nsteps and hook_fp is not None:
                        hook_fp()
                    if step < nsteps:
                        kt, hloc = divmod(step, 2)
                        hl = hA if hloc == 0 else hB
                        # per-head [128,1024] psum tile, double-buffered:
                        # exp of step s overlaps scores of step s+1
                        sc = psA.tile([128, 1024], F32, tag="sc")
                        for qch in range(2):
                            nc.tensor.matmul(
                                sc[:, qch * 512:(qch + 1) * 512],
                                lhsT=KT_ap(hl, kt),
                                rhs=QT_full(hl, qch),
                                start=True, stop=True)
                        w = wT_p.tile([128, 1024], F32R, tag="wT")
                        nc.scalar.activation(w[:], sc[:],
                                             mybir.ActivationFunctionType.Exp,
                                             scale=0.125)
                        if use_mask:
                            # multiply by the 0/1 keep-mask (pen[k, q]) with
                            # the same s16-major q permutation as wT columns
                            pap = pen_t(kt).rearrange("p (q s) -> p s q", s=16)
                            nc.vector.tensor_tensor(
                                w[:].rearrange("p (s q) -> p s q", s=16),
                                w[:].rearrange("p (s q) -> p s q", s=16),
                                pap, op=mybir.AluOpType.mult)
                        wstash[step] = w
                    if step >= lag:
                        kt, hloc = divmod(step - lag, 2)
                        hl = hA if hloc == 0 else hB
                        w = wstash.pop(step - lag)
                        for qch in range(2):
                            i = hloc + 2 * qch
                            if kt == 0:
                                pv[i] = psB.tile([65, 512], F32, tag="ps1",
                                                 name="pv")
                            nc.tensor.matmul(
                                pv[i][:],
                                lhsT=VA[kt][:, hl * 65:hl * 65 + 65],
                                rhs=w[:, qch * 512:(qch + 1) * 512],
                                start=(kt == 0), stop=(kt == 7))
                return pv

            def tail_norm(p, pv):
                hA, hB = 2 * p, 2 * p + 1
                # normalize + shuffle into final-projection lhsT layout
                lh = lh_p.tile([128, 1024], BF16)
                for hloc, hl in enumerate((hA, hB)):
                    for qch in range(2):
                        i = hloc + 2 * qch
                        nc.vector.reciprocal(
                            rc[0:1, qch * 512:(qch + 1) * 512], pv[i][64:65, :])
                    nc.gpsimd.partition_broadcast(rcb[:], rc[:])
                    rcv = rcb[:].rearrange("p (s q) -> p s q", s=16)
                    for qch in range(2):
                        i = hloc + 2 * qch
                        src = pv[i][0:64, :].rearrange("p (s q) -> p s q", s=8)
                        for par, off in ((0, 0), (1, 64)):  # even/odd s16
                            # lh layout: [part, (ct 8)(head 2)(q16 64)] so the
                            # final matmul's lhsT tile ct is one contiguous
                            # 128-col block (walrus: stationary AP needs a
                            # single free dim)
                            dst = lh[off:off + 64, :].rearrange(
                                "p (c m) -> p c m", c=8
                            )[:, qch * 4:(qch + 1) * 4,
                              hloc * 64:(hloc + 1) * 64]
                            nc.vector.tensor_tensor(
                                dst,
                                src[:, par::2, :],
                                rcv[:, qch * 8 + par:qch * 8 + 8:2, :],
                                op=mybir.AluOpType.mult)

                return lh

            def tail_fp(p, lh):
                # final projection: out rows p*128 .. p*128+128; ct-outer
                # with both ot psum tiles live (each lh chunk feeds 2
                # consecutive matmuls)
                ob = outp.tile([128, 1024], F32)
                fp = [psB.tile([128, 512], F32, tag="ps1", name="fp")
                      for _ in range(2)]
                for ct in range(8):
                    for ot in range(2):
                        nc.tensor.matmul(
                            fp[ot][:],
                            lhsT=lh[:, ct * 128:(ct + 1) * 128],
                            rhs=wo_t(ct)[:, ot * 512:(ot + 1) * 512],
                            start=(ct == 0), stop=(ct == 7))
                for ot in range(2):
                    nc.vector.tensor_copy(
                        ob[:, ot * 512:(ot + 1) * 512], fp[ot][:])
                nc.sync.dma_start(out[p * 128:(p + 1) * 128, :], ob[:])

            # software-pipeline: pair p-1's norm (DVE) is emitted in p-1's
            # own drain; the fp chains ride pair p's drain via the hook
            pending = None
            for p in range(4):
                hf = None
                if pending is not None:
                    pp, plh = pending

                    def hf(pp=pp, plh=plh):
                        tail_fp(pp, plh)

                pv = attention(p, hf, PV_LAG)
                pending = (p, tail_norm(p, pv))
            pp, plh = pending
            tail_fp(pp, plh)

          if loop_n is None:
              emit_body()
          else:
              with tc.For_i(0, loop_n):
                  emit_body()

    nc.compile()
    return nc


def make_in_maps(query, key, value, mask, Wq, bq, Wk, bk, Wv, bv, Wo,
                 pen_b=None):
    woT = np.ascontiguousarray(Wo.T).astype(BF16_NP)
    maps = []
    for c in range(8):
        b, hf = c // 2, c % 2
        sl = slice(hf * JC, (hf + 1) * JC)
        m = {
            "xqT": np.ascontiguousarray(query[b].T).astype(BF16_NP),
            "xkT": np.ascontiguousarray(key[b].T).astype(BF16_NP),
            "xvT": np.ascontiguousarray(value[b].T).astype(BF16_NP),
            "wqT": np.ascontiguousarray(Wq[sl].T).astype(BF16_NP),
            "wkT": np.ascontiguousarray(Wk[sl].T).astype(BF16_NP),
            "wvT": np.ascontiguousarray(Wv[sl].T).astype(BF16_NP),
            "bq_col": np.ascontiguousarray(bq[sl].reshape(4, 128).T),
            "bk_col": np.ascontiguousarray(bk[sl].reshape(4, 128).T),
            "bv_bc": np.ascontiguousarray(
                np.broadcast_to(bv[sl].reshape(1, JC), (128, JC))),
            "woT": woT,
        }
        if pen_b is not None:
            m["pen"] = pen_b[b]
        maps.append(m)
    return maps


def kernel(query, key, value, mask, Wq, bq, Wk, bk, Wv, bv, Wo):
    query = np.asarray(query, np.float32)
    key = np.asarray(key, np.float32)
    value = np.asarray(value, np.float32)
    mask = np.asarray(mask, np.float32)

    m2d = mask[0]  # [B, S, 64]
    mm = np.stack([m2d[b] @ m2d[b].T for b in range(B)])  # [B, S, S]
    use_mask = bool((mm == 0).any())
    pen_b = None
    if use_mask:
        pen_b = np.where(mm == 0, np.float32(0.0), np.float32(1.0))
        pen_b = np.ascontiguousarray(pen_b, np.float32)

    if use_mask not in _cached:
        _cached[use_mask] = build_program(use_mask)
    nc = _cached[use_mask]

    in_maps = make_in_maps(query, key, value, mask,
                           np.asarray(Wq, np.float32), np.asarray(bq, np.float32),
                           np.asarray(Wk, np.float32), np.asarray(bk, np.float32),
                           np.asarray(Wv, np.float32), np.asarray(bv, np.float32),
                           np.asarray(Wo, np.float32), pen_b)
    res = run_bass_kernel_spmd(nc, in_maps, list(range(8)))

    out = np.empty((B, S, D), np.float32)
    for c in range(8):
        b, hf = c // 2, c % 2
        out[b, hf * JC:(hf + 1) * JC, :] = res.results[c]["out"]
    return out
